# revision 13
# baseline (speedup 1.0000x reference)
"""Continual-attention Trainium2 kernel (8 NeuronCores, SPMD).

Sharding: core c -> batch b = c//2, head-group g = c%2 (4 heads each).
Per (b,h) computes S^T[k,q] = K Q^T via PE (fp16, zero-padded to 128
contraction rows).  Scores are exponentiated in [128,<=1024] PSUM pair-
groups on one of two engines so exp runs concurrently on both:
  - ACT: exp with fused 1/sqrt(d) scale -> pt fp16, plus DVE 0/1
    diagonal-mask multiplies on 128-col diag sub-blocks, or
  - DVE: fused Schraudolph exp + mask in ONE scalar_tensor_tensor:
    y_i16 = round(S*A0 + btile); btile carries B0 (allowed) or B0-50000
    (masked -> int16 saturates -> bitcast fp16 -0.0).  Used for the gq3
    per-batch test-train mask (mttb, data) and the test-chunk mask
    (chunkB2, const).  pt = int16 tile bitcast fp16 (~1.8% rms PL-exp).
Heads are processed as two zipped streams in anti-phase (stream A: head
h gq0->gq3, stream B: head h+1 gq3,gq0->gq2) so ACT-heavy and DVE-heavy
q-groups overlap.  O^T[d,q] (+ denominator as a 65th row via a ones
column in V) accumulates on PE; PSUM->SBUF output copies alternate
ACT/DVE; normalization + final transpose happen on host.
"""

import sys

sys.path.insert(0, "/opt/trn_rl_repo")

import numpy as np

B, L, H, D = 4, 2048, 8, 64
TRAIN = 1536
TEST = L - TRAIN            # 512
NCH = 64                    # test chunks
CH = TEST // NCH            # 8
HPC = 4                     # heads per core
NCORES = 8
KT = L // 128               # 16 k-tiles

# Schraudolph fp16-bitcast exp constants (applied to RAW logits; the 1/8
# scale is folded into A0).  sigma=60 centers the piecewise-linear error.
A0 = float((2.0**10) / np.log(2.0) * 0.125)
B0 = 15.0 * 1024 - 60.0
BMASK = B0 - 50000.0        # forces int16 saturation -> fp16 -0.0

LAST_RESULT = None
_PROG = None


def _split_multi_waits(nc, mybir):
    """This container's walrus accepts at most one semaphore wait per
    instruction; Tile's tail drains can carry several. Hoist extras onto
    NoOps inserted immediately before, on the same engine."""
    for f in nc.m.functions:
        for bb in f.blocks:
            insts = list(bb.instructions)
            out = []
            changed = False
            for inst in insts:
                si = inst.sync_info
                if si is not None and len(si.on_wait) > 1:
                    waits = list(si.on_wait)
                    for w in waits[:-1]:
                        nop = mybir.InstNoOp(
                            name=f"waitnop-{nc.next_id()}", ins=[], outs=[]
                        )
                        nop.engine = inst.engine
                        nop.sync_info = mybir.SyncInfo(on_wait=[w], on_update=[])
                        out.append(nop)
                    inst.sync_info = mybir.SyncInfo(
                        on_wait=[waits[-1]], on_update=list(si.on_update)
                    )
                    changed = True
                out.append(inst)
            if changed:
                bb.instructions = out


def _geom(gq, kp):
    """(off, w): query-column offset within the 512-wide q-group and width
    of the causally-needed slice for k-tile kp."""
    if kp <= 11:
        off = max(0, 128 * kp - 512 * gq)
        return off, 512 - off
    off = 128 * (kp - 12)
    return off, 128


def _groups_for_gq(gq):
    """Pair-groups per q-group: (kps, engine, bsrc) where engine 'act'
    uses exact exp (+ DVE diag muls), 'dve' uses the fused schraudolph
    STT with bsrc in {'mttb', 'chunk2'}."""
    if gq < 3:
        n = 4 * (gq + 1)
        return [
            ([k, k + 1], "act", None) for k in range(0, n, 2)
        ]
    return [
        ([0, 1], "dve", "mttb"),
        ([2, 3], "dve", "mttb"),
        ([4, 5], "dve", "mttb"),
        ([6, 7], "dve", "mttb"),
        ([8, 9], "dve", "mttb"),
        ([10, 11], "dve", "mttb"),
        ([12, 13], "dve", "chunk2"),
        ([14, 15], "dve", "chunk2"),
    ]


def _build_program():
    import concourse.bass as bass
    import concourse.mybir as mybir
    import concourse.tile as tile

    f32 = mybir.dt.float32
    fp16 = mybir.dt.float16
    i16 = mybir.dt.int16
    Exp = mybir.ActivationFunctionType.Exp
    Alu = mybir.AluOpType

    nc = bass.Bass()

    qt_d = nc.dram_tensor("qt", [HPC, 128, L], fp16, kind="ExternalInput")
    kt_d = nc.dram_tensor("kt", [HPC, 128, L], fp16, kind="ExternalInput")
    vw_d = nc.dram_tensor("vw", [HPC, 128, KT * 65], fp16, kind="ExternalInput")
    mttb_d = nc.dram_tensor("mttb", [128, 12 * 512], fp16, kind="ExternalInput")
    mdiag_d = nc.dram_tensor("mdiag", [128, 128], fp16, kind="ExternalInput")
    chunk2_d = nc.dram_tensor("chunk2", [128, 256], fp16, kind="ExternalInput")
    ot_d = nc.dram_tensor("ot", [HPC, 65, L], f32, kind="ExternalOutput")

    with tile.TileContext(nc) as tc:
        with (
            tc.tile_pool(name="consts", bufs=1) as consts,
            tc.tile_pool(name="heads", bufs=4) as heads,
            tc.tile_pool(name="ptp", bufs=8) as ptp,
            tc.tile_pool(name="osbp", bufs=4) as osbp,
            tc.tile_pool(name="spp", bufs=3, space="PSUM") as spp,
            tc.tile_pool(name="avp", bufs=2, space="PSUM") as avp,
        ):
            mdiag_sb = consts.tile([128, 128], fp16)
            nc.sync.dma_start(out=mdiag_sb, in_=mdiag_d.ap())
            chunk2_sb = consts.tile([128, 256], fp16)
            mttb_sb = consts.tile([128, 12 * 512], fp16)

            hsb = [None] * HPC

            def load_heads(ha, hb, with_consts):
                """Issue DMAs for a head pair in compute-need order:
                stream A consumes head ha gq0->gq3 (kt/qt columns from the
                front), stream B consumes head hb gq3 first (qt tail +
                mttb).  Fine chunks let compute start early."""
                tiles = {}
                for h in (ha, hb):
                    qt_sb = heads.tile([128, L], fp16, tag="qt")
                    kt_sb = heads.tile([128, L], fp16, tag="kt")
                    vw_sb = heads.tile([128, KT, 65], fp16, tag="vw")
                    hsb[h] = (qt_sb, kt_sb, vw_sb)
                    tiles[h] = (qt_sb, kt_sb, vw_sb)

                def cp(t, d, h, c0, c1):
                    nc.sync.dma_start(
                        out=t[:, c0:c1], in_=d.ap()[h][:, c0:c1]
                    )

                qa, ka, va = tiles[ha]
                qb, kb, vb = tiles[hb]
                cp(ka, kt_d, ha, 0, 256)
                cp(qa, qt_d, ha, 0, 512)
                cp(kb, kt_d, hb, 0, 768)
                cp(qb, qt_d, hb, 1536, 2048)
                cp(ka, kt_d, ha, 256, 512)
                if with_consts:
                    nc.sync.dma_start(
                        out=mttb_sb[:, 0:1536], in_=mttb_d.ap()[:, 0:1536]
                    )
                nc.sync.dma_start(
                    out=va, in_=vw_d.ap()[ha].rearrange("p (t c) -> p t c", t=KT)
                )
                nc.sync.dma_start(
                    out=vb, in_=vw_d.ap()[hb].rearrange("p (t c) -> p t c", t=KT)
                )
                if with_consts:
                    nc.sync.dma_start(
                        out=mttb_sb[:, 1536:3584], in_=mttb_d.ap()[:, 1536:3584]
                    )
                cp(ka, kt_d, ha, 512, 1024)
                cp(qa, qt_d, ha, 512, 1024)
                cp(kb, kt_d, hb, 768, 1536)
                if with_consts:
                    nc.sync.dma_start(
                        out=mttb_sb[:, 3584:6144], in_=mttb_d.ap()[:, 3584:6144]
                    )
                    nc.sync.dma_start(out=chunk2_sb, in_=chunk2_d.ap())
                cp(ka, kt_d, ha, 1024, 2048)
                cp(qa, qt_d, ha, 1024, 2048)
                cp(kb, kt_d, hb, 1536, 2048)
                cp(qb, qt_d, hb, 0, 1536)

            ncopies = [0]

            class Unit:
                """One (head, q-group): S -> exp -> AV -> copy out."""

                def __init__(self, h, gq):
                    self.h, self.gq = h, gq
                    self.qt, self.kt, self.vw = hsb[h]
                    self.groups = list(_groups_for_gq(gq))
                    self.last_kp = self.groups[-1][0][-1]
                    self.av = None
                    self.gi = 0
                    self.pending = []
                    self.PIPE = 2

                def _emit_av(self, args):
                    kp, pt, pos, off, w, start, stop = args
                    nc.tensor.matmul(
                        self.av[:65, off : off + w],
                        lhsT=self.vw[:, kp, :],
                        rhs=pt[:, pos : pos + w],
                        start=start,
                        stop=stop,
                        skip_group_check=True,
                    )

                def step(self):
                    """Emit one group; return False when unit is done."""
                    if self.av is None:
                        self.av = avp.tile([128, 512], f32, tag="av")
                    if self.gi < len(self.groups):
                        kps, eng, bsrc = self.groups[self.gi]
                        self.gi += 1
                        gq = self.gq
                        geo = [_geom(gq, kp) for kp in kps]
                        poss = []
                        cur = 0
                        for _, w in geo:
                            if (cur % 512) + w > 512:
                                cur = (cur // 512 + 1) * 512
                            poss.append(cur)
                            cur += w
                        span = cur

                        sp = spp.tile([128, 1024], f32, tag="sp")
                        for kp, (off, w), pos in zip(kps, geo, poss):
                            qs = 512 * gq + off
                            nc.tensor.matmul(
                                sp[:, pos : pos + w],
                                lhsT=self.kt[:, 128 * kp : 128 * kp + 128],
                                rhs=self.qt[:, qs : qs + w],
                                start=True,
                                stop=True,
                                skip_group_check=True,
                            )

                        pt = ptp.tile([128, 1024], fp16, tag="pt")
                        if eng == "dve":
                            if bsrc == "mttb":
                                bt = mttb_sb[:, 512 * kps[0] : 512 * kps[0] + span]
                            else:
                                bt = chunk2_sb[:, 0:span]
                            nc.vector.scalar_tensor_tensor(
                                pt.bitcast(i16)[:, 0:span],
                                sp[:, 0:span],
                                A0,
                                bt,
                                op0=Alu.mult,
                                op1=Alu.add,
                            )
                        else:
                            nc.scalar.activation(
                                pt[:, 0:span], sp[:, 0:span], Exp, scale=0.125
                            )
                            for kp, pos in zip(kps, poss):
                                if kp <= 11 and 128 * kp >= 512 * gq:
                                    nc.vector.tensor_mul(
                                        pt[:, pos : pos + 128],
                                        pt[:, pos : pos + 128],
                                        mdiag_sb,
                                    )

                        self.pending.append(
                            [
                                (kp, pt, pos, off, w, kp == 0, kp == self.last_kp)
                                for kp, (off, w), pos in zip(kps, geo, poss)
                            ]
                        )
                        if len(self.pending) > self.PIPE:
                            for args in self.pending.pop(0):
                                self._emit_av(args)
                        return True

                    if self.pending:
                        for grp in self.pending:
                            for args in grp:
                                self._emit_av(args)
                        self.pending = []
                        osb = osbp.tile([65, 512], f32)
                        if ncopies[0] % 2 == 0:
                            nc.vector.tensor_copy(osb, self.av[:65, :])
                        else:
                            nc.scalar.copy(osb, self.av[:65, :])
                        ncopies[0] += 1
                        nc.sync.dma_start(
                            out=ot_d.ap()[self.h][
                                :, 512 * self.gq : 512 * self.gq + 512
                            ],
                            in_=osb,
                        )
                    return False

            # Two anti-phase streams per head pair: A = gq0..gq3 of head
            # ha (ACT-heavy first), B = gq3,gq0..gq2 of head hb.
            for ha, hb in ((0, 1), (2, 3)):
                load_heads(ha, hb, with_consts=(ha == 0))
                qa = [Unit(ha, gq) for gq in (0, 1, 2, 3)]
                qb = [Unit(hb, gq) for gq in (3, 0, 1, 2)]
                qa.reverse()
                qb.reverse()
                cura = curb = None
                while qa or qb or cura or curb:
                    if cura is None and qa:
                        cura = qa.pop()
                    if cura is not None and not cura.step():
                        cura = None
                    if curb is None and qb:
                        curb = qb.pop()
                    if curb is not None and not curb.step():
                        curb = None

    import concourse.mybir as mybir_mod

    _split_multi_waits(nc, mybir_mod)
    return nc


def _host_inputs(queries, keys, values, attach):
    """Build per-core input maps (host-side layout prep)."""
    f16 = np.float16
    p = np.arange(128)
    f = np.arange(128)
    mdiag = np.where(f[None, :] >= p[:, None], 1.0, 0.0).astype(np.float32)
    chunkallow = (p[:, None] // CH == f[None, :] // CH) & (
        p[:, None] <= f[None, :]
    )
    chunkB = np.where(chunkallow, B0, BMASK).astype(np.float32)
    chunk2 = np.concatenate([chunkB, chunkB], axis=1)  # [128, 256]

    in_maps = []
    for c in range(NCORES):
        b, g = divmod(c, 2)
        hs = slice(HPC * g, HPC * (g + 1))
        q = queries[b][:, hs, :]          # [L, 4, D]
        k = keys[b][:, hs, :]
        v = values[b][:, hs, :]
        qt = np.zeros((HPC, 128, L), np.float32)
        qt[:, :D, :] = q.transpose(1, 2, 0)
        kt = np.zeros((HPC, 128, L), np.float32)
        kt[:, :D, :] = k.transpose(1, 2, 0)
        vw = np.empty((HPC, L, 65), np.float32)
        vw[:, :, :64] = v.transpose(1, 0, 2)
        vw[:, :, 64] = 1.0
        vw = np.ascontiguousarray(
            vw.reshape(HPC, KT, 128, 65).transpose(0, 2, 1, 3).reshape(HPC, 128, KT * 65)
        )
        kg = (np.arange(12)[:, None] * 128 + np.arange(128)[None, :])  # [12,128]
        thr = attach[b][np.arange(TEST) // CH]                          # [512]
        mttb = np.where(kg[:, :, None] <= thr[None, None, :], B0, BMASK)
        mttb = np.ascontiguousarray(
            mttb.transpose(1, 0, 2).reshape(128, 12 * 512)
        )
        in_maps.append(
            {
                "qt": qt.astype(f16),
                "kt": kt.astype(f16),
                "vw": vw.astype(f16),
                "mttb": mttb.astype(f16),
                "mdiag": mdiag.astype(f16),
                "chunk2": chunk2.astype(f16),
            }
        )
    return in_maps


def kernel(queries, keys, values, attach_test_after, train_len):
    global LAST_RESULT, _PROG
    import os

    queries = np.asarray(queries, dtype=np.float32)
    keys = np.asarray(keys, dtype=np.float32)
    values = np.asarray(values, dtype=np.float32)
    attach = np.asarray(attach_test_after).astype(np.int64)
    tl = int(np.asarray(train_len))
    assert queries.shape == (B, L, H, D), queries.shape
    assert tl == TRAIN and attach.shape == (B, NCH)

    from concourse.bass_utils import run_bass_kernel_spmd

    if _PROG is None:
        _PROG = _build_program()

    in_maps = _host_inputs(queries, keys, values, attach)
    trace = bool(int(os.environ.get("KERNEL_TRACE", "0")))
    res = run_bass_kernel_spmd(
        _PROG, in_maps, core_ids=list(range(NCORES)), trace=trace
    )
    LAST_RESULT = res

    out = np.empty((B, L, H * D), np.float32)
    for c in range(NCORES):
        b, g = divmod(c, 2)
        ot = res.results[c]["ot"]                     # [4, 65, L]
        o = ot[:, :64, :] / ot[:, 64:65, :]           # [4, 64, L]
        out[b, :, 256 * g : 256 * (g + 1)] = (
            o.transpose(2, 0, 1).reshape(L, HPC * D)
        )
    return out


# revision 14
# speedup vs baseline: 1.0357x; 1.0357x over previous
"""Continual-attention Trainium2 kernel (8 NeuronCores, SPMD).

Sharding: core c -> batch b = c//2, head-group g = c%2 (4 heads each).
Per (b,h) computes S^T[k,q] = K Q^T via PE (fp16, zero-padded to 128
contraction rows).  Scores are exponentiated in [128,<=1024] PSUM pair-
groups on one of two engines so exp runs concurrently on both:
  - ACT: exp with fused 1/sqrt(d) scale -> pt fp16, plus DVE 0/1
    diagonal-mask multiplies on 128-col diag sub-blocks, or
  - DVE: fused Schraudolph exp + mask in ONE scalar_tensor_tensor:
    y_i16 = round(S*A0 + btile); btile carries B0 (allowed) or B0-50000
    (masked -> int16 saturates -> bitcast fp16 -0.0).  Used for the gq3
    per-batch test-train mask (mttb, data) and the test-chunk mask
    (chunkB2, const).  pt = int16 tile bitcast fp16 (~1.8% rms PL-exp).
Heads are processed as two zipped streams in anti-phase (stream A: head
h gq0->gq3, stream B: head h+1 gq3,gq0->gq2) so ACT-heavy and DVE-heavy
q-groups overlap.  O^T[d,q] (+ denominator as a 65th row via a ones
column in V) accumulates on PE; PSUM->SBUF output copies alternate
ACT/DVE; normalization + final transpose happen on host.
"""

import sys

sys.path.insert(0, "/opt/trn_rl_repo")

import numpy as np

B, L, H, D = 4, 2048, 8, 64
TRAIN = 1536
TEST = L - TRAIN            # 512
NCH = 64                    # test chunks
CH = TEST // NCH            # 8
HPC = 4                     # heads per core
NCORES = 8
KT = L // 128               # 16 k-tiles

# Schraudolph fp16-bitcast exp constants (applied to RAW logits; the 1/8
# scale is folded into A0).  sigma=60 centers the piecewise-linear error.
A0 = float((2.0**10) / np.log(2.0) * 0.125)
B0 = 15.0 * 1024 - 60.0
BMASK = B0 - 50000.0        # forces int16 saturation -> fp16 -0.0

LAST_RESULT = None
_PROG = None


def _split_multi_waits(nc, mybir):
    """This container's walrus accepts at most one semaphore wait per
    instruction; Tile's tail drains can carry several. Hoist extras onto
    NoOps inserted immediately before, on the same engine."""
    for f in nc.m.functions:
        for bb in f.blocks:
            insts = list(bb.instructions)
            out = []
            changed = False
            for inst in insts:
                si = inst.sync_info
                if si is not None and len(si.on_wait) > 1:
                    waits = list(si.on_wait)
                    for w in waits[:-1]:
                        nop = mybir.InstNoOp(
                            name=f"waitnop-{nc.next_id()}", ins=[], outs=[]
                        )
                        nop.engine = inst.engine
                        nop.sync_info = mybir.SyncInfo(on_wait=[w], on_update=[])
                        out.append(nop)
                    inst.sync_info = mybir.SyncInfo(
                        on_wait=[waits[-1]], on_update=list(si.on_update)
                    )
                    changed = True
                out.append(inst)
            if changed:
                bb.instructions = out


def _geom(gq, kp):
    """(off, w): query-column offset within the 512-wide q-group and width
    of the causally-needed slice for k-tile kp."""
    if kp <= 11:
        off = max(0, 128 * kp - 512 * gq)
        return off, 512 - off
    off = 128 * (kp - 12)
    return off, 128


def _groups_for_gq(gq):
    """Pair-groups per q-group: (kps, engine, bsrc) where engine 'act'
    uses exact exp (+ DVE diag muls), 'dve' uses the fused schraudolph
    STT with bsrc in {'mttb', 'chunk2'}."""
    if gq < 3:
        n = 4 * (gq + 1)
        return [
            ([k, k + 1], "act", None) for k in range(0, n, 2)
        ]
    return [
        ([0, 1], "dve", "mttb"),
        ([2, 3], "dve", "mttb"),
        ([4, 5], "dve", "mttb"),
        ([6, 7], "dve", "mttb"),
        ([8, 9], "dve", "mttb"),
        ([10, 11], "dve", "mttb"),
        ([12, 13], "dve", "chunk2"),
        ([14, 15], "dve", "chunk2"),
    ]


def _build_program():
    import concourse.bass as bass
    import concourse.mybir as mybir
    import concourse.tile as tile

    f32 = mybir.dt.float32
    fp16 = mybir.dt.float16
    i16 = mybir.dt.int16
    Exp = mybir.ActivationFunctionType.Exp
    Alu = mybir.AluOpType

    nc = bass.Bass()

    qt_d = nc.dram_tensor("qt", [HPC, 128, L], fp16, kind="ExternalInput")
    kt_d = nc.dram_tensor("kt", [HPC, 128, L], fp16, kind="ExternalInput")
    vw_d = nc.dram_tensor("vw", [HPC, 128, KT * 65], fp16, kind="ExternalInput")
    mttb_d = nc.dram_tensor("mttb", [128, 12 * 512], fp16, kind="ExternalInput")
    mdiag_d = nc.dram_tensor("mdiag", [128, 128], fp16, kind="ExternalInput")
    chunk2_d = nc.dram_tensor("chunk2", [128, 256], fp16, kind="ExternalInput")
    ot_d = nc.dram_tensor("ot", [HPC, 65, L], f32, kind="ExternalOutput")

    with tile.TileContext(nc) as tc:
        with (
            tc.tile_pool(name="consts", bufs=1) as consts,
            tc.tile_pool(name="heads", bufs=4) as heads,
            tc.tile_pool(name="ptp", bufs=8) as ptp,
            tc.tile_pool(name="osbp", bufs=4) as osbp,
            tc.tile_pool(name="spp", bufs=3, space="PSUM") as spp,
            tc.tile_pool(name="avp", bufs=2, space="PSUM") as avp,
        ):
            mdiag_sb = consts.tile([128, 128], fp16)
            nc.sync.dma_start(out=mdiag_sb, in_=mdiag_d.ap())
            chunk2_sb = consts.tile([128, 256], fp16)
            mttb_sb = consts.tile([128, 12 * 512], fp16)

            hsb = [None] * HPC

            def load_heads(ha, hb, with_consts):
                """Issue DMAs for a head pair in compute-need order:
                stream A consumes head ha gq0->gq3 (kt/qt columns from the
                front), stream B consumes head hb gq3 first (qt tail +
                mttb).  Fine chunks let compute start early."""
                tiles = {}
                for h in (ha, hb):
                    qt_sb = heads.tile([128, L], fp16, tag="qt")
                    kt_sb = heads.tile([128, L], fp16, tag="kt")
                    vw_sb = heads.tile([128, KT, 65], fp16, tag="vw")
                    hsb[h] = (qt_sb, kt_sb, vw_sb)
                    tiles[h] = (qt_sb, kt_sb, vw_sb)

                def cp(t, d, h, c0, c1):
                    nc.sync.dma_start(
                        out=t[:, c0:c1], in_=d.ap()[h][:, c0:c1]
                    )

                qa, ka, va = tiles[ha]
                qb, kb, vb = tiles[hb]
                cp(ka, kt_d, ha, 0, 512)
                cp(qa, qt_d, ha, 0, 512)
                cp(kb, kt_d, hb, 0, 768)
                cp(qb, qt_d, hb, 1536, 2048)
                if with_consts:
                    nc.sync.dma_start(
                        out=mttb_sb[:, 0:1536], in_=mttb_d.ap()[:, 0:1536]
                    )
                nc.sync.dma_start(
                    out=va, in_=vw_d.ap()[ha].rearrange("p (t c) -> p t c", t=KT)
                )
                nc.sync.dma_start(
                    out=vb, in_=vw_d.ap()[hb].rearrange("p (t c) -> p t c", t=KT)
                )
                if with_consts:
                    nc.sync.dma_start(
                        out=mttb_sb[:, 1536:3584], in_=mttb_d.ap()[:, 1536:3584]
                    )
                cp(ka, kt_d, ha, 512, 1024)
                cp(qa, qt_d, ha, 512, 1024)
                cp(kb, kt_d, hb, 768, 1536)
                if with_consts:
                    nc.sync.dma_start(
                        out=mttb_sb[:, 3584:6144], in_=mttb_d.ap()[:, 3584:6144]
                    )
                    nc.sync.dma_start(out=chunk2_sb, in_=chunk2_d.ap())
                cp(ka, kt_d, ha, 1024, 2048)
                cp(qa, qt_d, ha, 1024, 2048)
                cp(kb, kt_d, hb, 1536, 2048)
                cp(qb, qt_d, hb, 0, 1536)

            ncopies = [0]

            class Unit:
                """One (head, q-group): S -> exp -> AV -> copy out."""

                def __init__(self, h, gq):
                    self.h, self.gq = h, gq
                    self.qt, self.kt, self.vw = hsb[h]
                    self.groups = list(_groups_for_gq(gq))
                    self.last_kp = self.groups[-1][0][-1]
                    self.av = None
                    self.gi = 0
                    self.pending = []
                    self.PIPE = 2

                def _emit_av(self, args):
                    kp, pt, pos, off, w, start, stop = args
                    nc.tensor.matmul(
                        self.av[:65, off : off + w],
                        lhsT=self.vw[:, kp, :],
                        rhs=pt[:, pos : pos + w],
                        start=start,
                        stop=stop,
                        skip_group_check=True,
                    )

                def step(self):
                    """Emit one group; return False when unit is done."""
                    if self.av is None:
                        self.av = avp.tile([128, 512], f32, tag="av")
                    if self.gi < len(self.groups):
                        kps, eng, bsrc = self.groups[self.gi]
                        self.gi += 1
                        gq = self.gq
                        geo = [_geom(gq, kp) for kp in kps]
                        poss = []
                        cur = 0
                        for _, w in geo:
                            if (cur % 512) + w > 512:
                                cur = (cur // 512 + 1) * 512
                            poss.append(cur)
                            cur += w
                        span = cur

                        sp = spp.tile([128, 1024], f32, tag="sp")
                        for kp, (off, w), pos in zip(kps, geo, poss):
                            qs = 512 * gq + off
                            nc.tensor.matmul(
                                sp[:, pos : pos + w],
                                lhsT=self.kt[:, 128 * kp : 128 * kp + 128],
                                rhs=self.qt[:, qs : qs + w],
                                start=True,
                                stop=True,
                                skip_group_check=True,
                            )

                        pt = ptp.tile([128, 1024], fp16, tag="pt")
                        if eng == "dve":
                            if bsrc == "mttb":
                                bt = mttb_sb[:, 512 * kps[0] : 512 * kps[0] + span]
                            else:
                                bt = chunk2_sb[:, 0:span]
                            nc.vector.scalar_tensor_tensor(
                                pt.bitcast(i16)[:, 0:span],
                                sp[:, 0:span],
                                A0,
                                bt,
                                op0=Alu.mult,
                                op1=Alu.add,
                            )
                        else:
                            nc.scalar.activation(
                                pt[:, 0:span], sp[:, 0:span], Exp, scale=0.125
                            )
                            for kp, pos in zip(kps, poss):
                                if kp <= 11 and 128 * kp >= 512 * gq:
                                    nc.vector.tensor_mul(
                                        pt[:, pos : pos + 128],
                                        pt[:, pos : pos + 128],
                                        mdiag_sb,
                                    )

                        self.pending.append(
                            [
                                (kp, pt, pos, off, w, kp == 0, kp == self.last_kp)
                                for kp, (off, w), pos in zip(kps, geo, poss)
                            ]
                        )
                        if len(self.pending) > self.PIPE:
                            for args in self.pending.pop(0):
                                self._emit_av(args)
                        return True

                    if self.pending:
                        for grp in self.pending:
                            for args in grp:
                                self._emit_av(args)
                        self.pending = []
                        osb = osbp.tile([65, 512], f32)
                        if ncopies[0] % 2 == 0:
                            nc.vector.tensor_copy(osb, self.av[:65, :])
                        else:
                            nc.scalar.copy(osb, self.av[:65, :])
                        ncopies[0] += 1
                        nc.sync.dma_start(
                            out=ot_d.ap()[self.h][
                                :, 512 * self.gq : 512 * self.gq + 512
                            ],
                            in_=osb,
                        )
                    return False

            # Two anti-phase streams per head pair: A = gq0..gq3 of head
            # ha (ACT-heavy first), B = gq3,gq0..gq2 of head hb.
            for ha, hb in ((0, 1), (2, 3)):
                load_heads(ha, hb, with_consts=(ha == 0))
                qa = [Unit(ha, gq) for gq in (0, 1, 2, 3)]
                qb = [Unit(hb, gq) for gq in (3, 0, 1, 2)]
                qa.reverse()
                qb.reverse()
                cura = curb = None
                while qa or qb or cura or curb:
                    if cura is None and qa:
                        cura = qa.pop()
                    if cura is not None and not cura.step():
                        cura = None
                    if curb is None and qb:
                        curb = qb.pop()
                    if curb is not None and not curb.step():
                        curb = None

    import concourse.mybir as mybir_mod

    _split_multi_waits(nc, mybir_mod)
    return nc


def _host_inputs(queries, keys, values, attach):
    """Build per-core input maps (host-side layout prep)."""
    f16 = np.float16
    p = np.arange(128)
    f = np.arange(128)
    mdiag = np.where(f[None, :] >= p[:, None], 1.0, 0.0).astype(np.float32)
    chunkallow = (p[:, None] // CH == f[None, :] // CH) & (
        p[:, None] <= f[None, :]
    )
    chunkB = np.where(chunkallow, B0, BMASK).astype(np.float32)
    chunk2 = np.concatenate([chunkB, chunkB], axis=1)  # [128, 256]

    in_maps = []
    for c in range(NCORES):
        b, g = divmod(c, 2)
        hs = slice(HPC * g, HPC * (g + 1))
        q = queries[b][:, hs, :]          # [L, 4, D]
        k = keys[b][:, hs, :]
        v = values[b][:, hs, :]
        qt = np.zeros((HPC, 128, L), np.float32)
        qt[:, :D, :] = q.transpose(1, 2, 0)
        kt = np.zeros((HPC, 128, L), np.float32)
        kt[:, :D, :] = k.transpose(1, 2, 0)
        vw = np.empty((HPC, L, 65), np.float32)
        vw[:, :, :64] = v.transpose(1, 0, 2)
        vw[:, :, 64] = 1.0
        vw = np.ascontiguousarray(
            vw.reshape(HPC, KT, 128, 65).transpose(0, 2, 1, 3).reshape(HPC, 128, KT * 65)
        )
        kg = (np.arange(12)[:, None] * 128 + np.arange(128)[None, :])  # [12,128]
        thr = attach[b][np.arange(TEST) // CH]                          # [512]
        mttb = np.where(kg[:, :, None] <= thr[None, None, :], B0, BMASK)
        mttb = np.ascontiguousarray(
            mttb.transpose(1, 0, 2).reshape(128, 12 * 512)
        )
        in_maps.append(
            {
                "qt": qt.astype(f16),
                "kt": kt.astype(f16),
                "vw": vw.astype(f16),
                "mttb": mttb.astype(f16),
                "mdiag": mdiag.astype(f16),
                "chunk2": chunk2.astype(f16),
            }
        )
    return in_maps


def kernel(queries, keys, values, attach_test_after, train_len):
    global LAST_RESULT, _PROG
    import os

    queries = np.asarray(queries, dtype=np.float32)
    keys = np.asarray(keys, dtype=np.float32)
    values = np.asarray(values, dtype=np.float32)
    attach = np.asarray(attach_test_after).astype(np.int64)
    tl = int(np.asarray(train_len))
    assert queries.shape == (B, L, H, D), queries.shape
    assert tl == TRAIN and attach.shape == (B, NCH)

    from concourse.bass_utils import run_bass_kernel_spmd

    if _PROG is None:
        _PROG = _build_program()

    in_maps = _host_inputs(queries, keys, values, attach)
    trace = bool(int(os.environ.get("KERNEL_TRACE", "0")))
    res = run_bass_kernel_spmd(
        _PROG, in_maps, core_ids=list(range(NCORES)), trace=trace
    )
    LAST_RESULT = res

    out = np.empty((B, L, H * D), np.float32)
    for c in range(NCORES):
        b, g = divmod(c, 2)
        ot = res.results[c]["ot"]                     # [4, 65, L]
        o = ot[:, :64, :] / ot[:, 64:65, :]           # [4, 64, L]
        out[b, :, 256 * g : 256 * (g + 1)] = (
            o.transpose(2, 0, 1).reshape(L, HPC * D)
        )
    return out
